# revision 19
# baseline (speedup 1.0000x reference)
"""Trainium2 Bass kernel for nn_Ensemble_SRN (MoE-routed ensemble of SRN MLPs).

Problem: N=131072 points in [-1,1]^3, a 2x2x2 grid of M=8 sub-model MLPs
(3 -> 128 -> 128 -> 128 -> 1, relu), each point evaluated by the sub-model
owning its grid cell (per the reference's routing as executed on this
backend, whose float->int32 convert rounds to nearest: points whose cell
index lands outside 0..7 match no sub-model and output 0).

Strategy: routed data-parallel with per-chunk expert weights. The host
computes each point's sub-model index with the same jax ops as the
reference (bit-identical routing), drops the no-expert points (y=0),
normalizes each point into its expert's local box, and packs the surviving
points into single-expert chunks of 1024 points (2 device tiles). Chunks
are dealt round-robin across the 8 NeuronCores for near-perfect load
balance (expert loads are highly skewed by the rounding behavior). Each
chunk's weight set streams from DRAM alongside its points, so any core can
process any mix of experts.

Device program (per core, identical SPMD, T tiles = T/2 chunks):
  per chunk (one weight set, two 512-pt tiles in [128, 1024] buffers):
  - L0: psum = W0aug.T @ xT (K=4, ones row folds b0) -> relu -> h1 (f32r)
  - L1: psum = W1.T @ h1 (K=128) -> relu+b1 -> h2
  - L2: psum = W2.T @ h2 (K=128) -> relu+b2 -> h3
  - L3: psum = W3.T @ h3 (M=1) col-packed: the 4 y-rows of a 4-tile group
    land on partitions 0/32/64/96 of one PSUM bank, drained by one
    partition-strided copy, then DMA'd out.
Relus alternate between the Scalar(ACT) and Vector(DVE) engines. Matmuls
run in float32r (full PE row rate, ~1.5e-4 relative error vs fp32).
b3 is added on the host during the gather.
"""

import time

import numpy as np

import concourse.bass as bass
import concourse.mybir as mybir
import concourse.tile as tile
from concourse import bacc
from concourse.bass_utils import run_bass_kernel_spmd

N = 131072
M = 8
H = 128
GRID = (2, 2, 2)
NCORES = 8

PT = 512       # points per tile
CH = 2 * PT    # points per single-expert chunk

F32 = mybir.dt.float32
F32R = mybir.dt.float32r
RELU = mybir.ActivationFunctionType.Relu
COPY = mybir.ActivationFunctionType.Copy
BF16 = mybir.dt.bfloat16
ALU_ADD = mybir.AluOpType.add
ALU_MAX = mybir.AluOpType.max

_programs = {}  # n_chunks -> compiled Bacc


def _build_program(NC_CHUNKS):
    """NC_CHUNKS single-expert chunks of CH points; must be even (groups of
    two chunks share one y PSUM bank via col-packing)."""
    assert NC_CHUNKS % 2 == 0
    nc = bacc.Bacc("TRN2", target_bir_lowering=False, debug=False)

    xt_d = nc.dram_tensor("xt", [NC_CHUNKS, 4, CH], F32R, kind="ExternalInput")
    w0_d = nc.dram_tensor("w0", [NC_CHUNKS, 4, H], F32R, kind="ExternalInput")
    w1_d = nc.dram_tensor("w1", [NC_CHUNKS, H, H], F32R, kind="ExternalInput")
    w2_d = nc.dram_tensor("w2", [NC_CHUNKS, H, H], F32R, kind="ExternalInput")
    w3_d = nc.dram_tensor("w3", [NC_CHUNKS, H, 32], F32R, kind="ExternalInput")
    b1_d = nc.dram_tensor("b1", [NC_CHUNKS, H, 1], F32, kind="ExternalInput")
    b2_d = nc.dram_tensor("b2", [NC_CHUNKS, H, 1], F32, kind="ExternalInput")
    y_d = nc.dram_tensor("y", [NC_CHUNKS, CH], F32, kind="ExternalOutput")

    halves = (slice(0, PT), slice(PT, CH))

    with tile.TileContext(nc) as tc:
        with (
            tc.tile_pool(name="xp", bufs=4) as xp,
            tc.tile_pool(name="wp", bufs=4) as wp,
            tc.tile_pool(name="bp", bufs=4) as bp,
            tc.tile_pool(name="hp", bufs=3) as hp,
            tc.tile_pool(name="yp", bufs=2) as yp,
            tc.tile_pool(name="psm", bufs=3, space=bass.MemorySpace.PSUM) as psm,
            tc.tile_pool(name="psy", bufs=1, space=bass.MemorySpace.PSUM) as psy,
        ):
            def relu(out, ps, b, on_act):
                if on_act:
                    nc.scalar.activation(out, ps, RELU,
                                         bias=0.0 if b is None else b)
                else:
                    nc.vector.tensor_scalar(out, ps,
                                            0.0 if b is None else b,
                                            0.0, ALU_ADD, ALU_MAX)

            for g in range(NC_CHUNKS // 2):
                for ci in range(2):
                    c = 2 * g + ci
                    x_c = xp.tile([4, CH], F32R, tag="x")
                    nc.sync.dma_start(x_c[:], xt_d[c])
                    w0_c = wp.tile([4, H], F32R, tag="w0")
                    nc.sync.dma_start(w0_c[:], w0_d[c])
                    w1_c = wp.tile([H, H], F32R, tag="w1")
                    nc.sync.dma_start(w1_c[:], w1_d[c])
                    w2_c = wp.tile([H, H], F32R, tag="w2")
                    nc.sync.dma_start(w2_c[:], w2_d[c])
                    w3_c = wp.tile([H, 32], F32R, tag="w3")
                    nc.sync.dma_start(w3_c[:], w3_d[c])
                    b1_c = bp.tile([H, 1], F32, tag="b1")
                    nc.sync.dma_start(b1_c[:], b1_d[c])
                    b2_c = bp.tile([H, 1], F32, tag="b2")
                    nc.sync.dma_start(b2_c[:], b2_d[c])

                    on_act = c % 2 == 0

                    ps0 = psm.tile([H, CH], F32, tag="mm", name=f"ps0_{c}")
                    for i in range(2):
                        nc.tensor.matmul(ps0[:, halves[i]], w0_c[:],
                                         x_c[:, halves[i]],
                                         start=True, stop=True)
                    h1 = hp.tile([H, CH], F32R, tag="h1", name=f"h1_{c}")
                    relu(h1[:], ps0[:], None, on_act)

                    ps1 = psm.tile([H, CH], F32, tag="mm", name=f"ps1_{c}")
                    for i in range(2):
                        nc.tensor.matmul(ps1[:, halves[i]], w1_c[:],
                                         h1[:, halves[i]],
                                         start=True, stop=True)
                    h2 = hp.tile([H, CH], F32R, tag="h2", name=f"h2_{c}")
                    relu(h2[:], ps1[:], b1_c[:], not on_act)

                    ps2 = psm.tile([H, CH], F32, tag="mm", name=f"ps2_{c}")
                    for i in range(2):
                        nc.tensor.matmul(ps2[:, halves[i]], w2_c[:],
                                         h2[:, halves[i]],
                                         start=True, stop=True)
                    h3 = hp.tile([H, CH], F32R, tag="h3", name=f"h3_{c}")
                    relu(h3[:], ps2[:], b2_c[:], on_act)

                    ps3 = psy.tile([1, CH], F32, tag="y", name=f"ps3_{c}")
                    for i in range(2):
                        nc.tensor.matmul(ps3[:, halves[i]],
                                         w3_c[:, 0:1], h3[:, halves[i]],
                                         start=True, stop=True)
                    y1 = yp.tile([1, CH], F32, tag="y4", name=f"y1_{c}")
                    if on_act:
                        nc.scalar.activation(y1[:], ps3[:], COPY)
                    else:
                        nc.vector.tensor_copy(y1[:], ps3[:])
                    nc.sync.dma_start(y_d[c], y1[:])

    nc.compile()
    return nc


def _rne11(v):
    """Round fp32 to float32r's 11-bit mantissa (round-to-nearest-even).

    The PE consumes f32r as pre-rounded fp32 bits; feeding unrounded
    values produces garbage on HW (verified empirically: the casting DMA
    rounds exactly like this)."""
    b = np.ascontiguousarray(v, dtype=np.float32).view(np.uint32).astype(np.uint64)
    add = np.uint64(0x7FF) + ((b >> np.uint64(12)) & np.uint64(1))
    out = ((b + add) >> np.uint64(12) << np.uint64(12)).astype(np.uint32)
    return out.view(np.float32).reshape(np.asarray(v).shape)


def _route(x):
    """Per-point sub-model index, replicating the reference as executed.

    Uses the same jax ops on the same default device as the reference: on
    this backend the float->int32 convert rounds to nearest (not truncate),
    so cell components can be 2, making idx range over 0..14. Points with
    idx > 7 match no sub-model in the reference scan and keep y = 0.
    """
    import jax.numpy as jnp

    xj = jnp.asarray(np.ascontiguousarray(x, dtype=np.float32))
    g = jnp.asarray(np.array(GRID, dtype=np.float32))
    u = (xj + 1.0) / (2.0 + 1e-6)
    cell = np.asarray((u[:, ::-1] * g).astype(jnp.int32))
    idx = cell[:, 0] + cell[:, 1] * GRID[0] + cell[:, 2] * (GRID[0] * GRID[1])
    return idx


def _run_with_retry(nc, in_maps, trace):
    last = None
    for attempt in range(3):
        try:
            return run_bass_kernel_spmd(
                nc, in_maps, core_ids=list(range(NCORES)), trace=trace
            )
        except Exception as e:  # transient NRT_EXEC_UNIT_UNRECOVERABLE
            last = e
            time.sleep(15 * (attempt + 1))
    raise last


def kernel(x, W0, b0, W1, b1, W2, b2, W3, b3, mins, maxs, _trace=False):
    x = np.ascontiguousarray(x, dtype=np.float32)
    W0 = np.ascontiguousarray(W0, dtype=np.float32)
    b0 = np.ascontiguousarray(b0, dtype=np.float32)
    W1 = np.ascontiguousarray(W1, dtype=np.float32)
    b1 = np.ascontiguousarray(b1, dtype=np.float32)
    W2 = np.ascontiguousarray(W2, dtype=np.float32)
    b2 = np.ascontiguousarray(b2, dtype=np.float32)
    W3 = np.ascontiguousarray(W3, dtype=np.float32)
    b3 = np.ascontiguousarray(b3, dtype=np.float32)
    mins = np.ascontiguousarray(mins, dtype=np.float32)
    maxs = np.ascontiguousarray(maxs, dtype=np.float32)

    n = x.shape[0]
    idx = _route(x)
    valid = (idx >= 0) & (idx < M)

    order = np.argsort(np.where(valid, idx, M), kind="stable")
    counts = np.bincount(idx[valid], minlength=M)[:M]
    starts = np.concatenate([[0], np.cumsum(counts)])

    # Chop each expert's run of points into single-expert chunks of CH
    # points (last chunk of each expert padded), then deal chunks across
    # cores round-robin, largest first, for load balance.
    chunks = []  # (expert, sel_indices)
    for e in range(M):
        sel = order[starts[e] : starts[e + 1]]
        for o in range(0, len(sel), CH):
            chunks.append((e, sel[o : o + CH]))
    chunks.sort(key=lambda t: -len(t[1]))
    per_core = [chunks[c::NCORES] for c in range(NCORES)]
    n_chunks = max(len(pc) for pc in per_core)
    n_chunks = max(2, n_chunks + (n_chunks % 2))  # even, >= 2

    w0aug = np.concatenate([W0, b0[:, None, :]], axis=1)  # [M, 4, H]
    scale = np.float32(2.0) / (maxs - mins)  # [M, 3]

    in_maps = []
    placements = []
    for c in range(NCORES):
        xt = np.zeros((n_chunks, 4, CH), dtype=np.float32)
        wt0 = np.zeros((n_chunks, 4, H), dtype=np.float32)
        wt1 = np.zeros((n_chunks, H, H), dtype=np.float32)
        wt2 = np.zeros((n_chunks, H, H), dtype=np.float32)
        wt3 = np.zeros((n_chunks, H, 32), dtype=np.float32)
        bt1 = np.zeros((n_chunks, H, 1), dtype=np.float32)
        bt2 = np.zeros((n_chunks, H, 1), dtype=np.float32)
        place = []
        for s, (e, sel) in enumerate(per_core[c]):
            xn = np.float32(-1.0) + (x[sel] - mins[e]) * scale[e]
            xt[s, :3, : len(sel)] = xn.T
            xt[s, 3, :] = 1.0
            wt0[s] = w0aug[e]
            wt1[s] = W1[e]
            wt2[s] = W2[e]
            wt3[s] = W3[e]
            bt1[s, :, 0] = b1[e]
            bt2[s, :, 0] = b2[e]
            place.append((e, sel, s))
        placements.append(place)
        in_maps.append(
            {"xt": _rne11(xt), "w0": _rne11(wt0), "w1": _rne11(wt1),
             "w2": _rne11(wt2), "w3": _rne11(wt3),
             "b1": bt1, "b2": bt2}
        )

    if n_chunks not in _programs:
        _programs[n_chunks] = _build_program(n_chunks)
    nc = _programs[n_chunks]

    res = _run_with_retry(nc, in_maps, _trace)

    y = np.zeros((n, 1), dtype=np.float32)
    for c in range(NCORES):
        yc = res.results[c]["y"].reshape(n_chunks, CH)
        for e, sel, s in placements[c]:
            y[sel, 0] = yc[s, : len(sel)] + b3[e, 0]

    if _trace:
        kernel._last_result = res
    return y


# revision 21
# speedup vs baseline: 1.0363x; 1.0363x over previous
"""Trainium2 Bass kernel for nn_Ensemble_SRN (MoE-routed ensemble of SRN MLPs).

Problem: N=131072 points in [-1,1]^3, a 2x2x2 grid of M=8 sub-model MLPs
(3 -> 128 -> 128 -> 128 -> 1, relu), each point evaluated by the sub-model
owning its grid cell (per the reference's routing as executed on this
backend, whose float->int32 convert rounds to nearest: points whose cell
index lands outside 0..7 match no sub-model and output 0).

Strategy: routed data-parallel with per-chunk expert weights. The host
computes each point's sub-model index with the same jax ops as the
reference (bit-identical routing), drops the no-expert points (y=0),
normalizes each point into its expert's local box, and packs the surviving
points into single-expert chunks of 1024 points (2 device tiles). Chunks
are dealt round-robin across the 8 NeuronCores for near-perfect load
balance (expert loads are highly skewed by the rounding behavior). Each
chunk's weight set streams from DRAM alongside its points, so any core can
process any mix of experts.

Device program (per core, identical SPMD, T tiles = T/2 chunks):
  per chunk (one weight set, two 512-pt tiles in [128, 1024] buffers):
  - L0: psum = W0aug.T @ xT (K=4, ones row folds b0) -> relu -> h1 (f32r)
  - L1: psum = W1.T @ h1 (K=128) -> relu+b1 -> h2
  - L2: psum = W2.T @ h2 (K=128) -> relu+b2 -> h3
  - L3: psum = W3.T @ h3 (M=1) col-packed: the 4 y-rows of a 4-tile group
    land on partitions 0/32/64/96 of one PSUM bank, drained by one
    partition-strided copy, then DMA'd out.
Relus alternate between the Scalar(ACT) and Vector(DVE) engines. Matmuls
run in float32r (full PE row rate, ~1.5e-4 relative error vs fp32).
b3 is added on the host during the gather.
"""

import time

import numpy as np

import concourse.bass as bass
import concourse.mybir as mybir
import concourse.tile as tile
from concourse import bacc
from concourse.bass_utils import run_bass_kernel_spmd

N = 131072
M = 8
H = 128
GRID = (2, 2, 2)
NCORES = 8

PT = 512       # points per tile
CH = 2 * PT    # points per single-expert chunk

F32 = mybir.dt.float32
F32R = mybir.dt.float32r
RELU = mybir.ActivationFunctionType.Relu
COPY = mybir.ActivationFunctionType.Copy
BF16 = mybir.dt.bfloat16
ALU_ADD = mybir.AluOpType.add
ALU_MAX = mybir.AluOpType.max

_programs = {}  # n_chunks -> compiled Bacc


def _build_program(NC_CHUNKS):
    """NC_CHUNKS single-expert chunks of CH points; must be even (groups of
    two chunks share one y PSUM bank via col-packing)."""
    assert NC_CHUNKS % 2 == 0
    nc = bacc.Bacc("TRN2", target_bir_lowering=False, debug=False)

    # xw: per-chunk [4, CH + H]: cols 0:CH = points x^T (3 coords + ones),
    # cols CH: = W0aug (K=4 partition dim shared with x).
    xw_d = nc.dram_tensor("xw", [NC_CHUNKS, 4, CH + H], F32R,
                          kind="ExternalInput")
    # pw: per-chunk packed [H, 259]: 0:128 W1 | 128:256 W2 | 256 b1 |
    # 257 b2 | 258 w3.
    pw_d = nc.dram_tensor("pw", [NC_CHUNKS, H, 259], F32R,
                          kind="ExternalInput")
    y_d = nc.dram_tensor("y", [NC_CHUNKS, CH], F32, kind="ExternalOutput")

    halves = (slice(0, PT), slice(PT, CH))

    with tile.TileContext(nc) as tc:
        with (
            tc.tile_pool(name="xp", bufs=4) as xp,
            tc.tile_pool(name="wp", bufs=4) as wp,
            tc.tile_pool(name="bp", bufs=4) as bp,
            tc.tile_pool(name="hp", bufs=3) as hp,
            tc.tile_pool(name="yp", bufs=2) as yp,
            tc.tile_pool(name="psm", bufs=3, space=bass.MemorySpace.PSUM) as psm,
            tc.tile_pool(name="psy", bufs=1, space=bass.MemorySpace.PSUM) as psy,
        ):
            def relu(out, ps, b, on_act):
                if on_act:
                    nc.scalar.activation(out, ps, RELU,
                                         bias=0.0 if b is None else b)
                else:
                    nc.vector.tensor_scalar(out, ps,
                                            0.0 if b is None else b,
                                            0.0, ALU_ADD, ALU_MAX)

            for g in range(NC_CHUNKS // 2):
                for ci in range(2):
                    c = 2 * g + ci
                    xw_c = xp.tile([4, CH + H], F32R, tag="x")
                    nc.gpsimd.dma_start(xw_c[:], xw_d[c])
                    pw_c = wp.tile([H, 259], F32R, tag="pw")
                    nc.sync.dma_start(pw_c[:], pw_d[c])
                    x_c = xw_c[:, 0:CH]
                    w0_c = xw_c[:, CH : CH + H]
                    w1_c = pw_c[:, 0:H]
                    w2_c = pw_c[:, H : 2 * H]
                    b1_c = pw_c[:, 256:257].bitcast(F32)
                    b2_c = pw_c[:, 257:258].bitcast(F32)
                    w3_c = pw_c[:, 258:259]

                    on_act = c % 2 == 0

                    ps0 = psm.tile([H, CH], F32, tag="mm", name=f"ps0_{c}")
                    for i in range(2):
                        nc.tensor.matmul(ps0[:, halves[i]], w0_c,
                                         x_c[:, halves[i]],
                                         start=True, stop=True)
                    h1 = hp.tile([H, CH], F32R, tag="h1", name=f"h1_{c}")
                    relu(h1[:], ps0[:], None, on_act)

                    ps1 = psm.tile([H, CH], F32, tag="mm", name=f"ps1_{c}")
                    for i in range(2):
                        nc.tensor.matmul(ps1[:, halves[i]], w1_c,
                                         h1[:, halves[i]],
                                         start=True, stop=True)
                    h2 = hp.tile([H, CH], F32R, tag="h2", name=f"h2_{c}")
                    relu(h2[:], ps1[:], b1_c, not on_act)

                    ps2 = psm.tile([H, CH], F32, tag="mm", name=f"ps2_{c}")
                    for i in range(2):
                        nc.tensor.matmul(ps2[:, halves[i]], w2_c,
                                         h2[:, halves[i]],
                                         start=True, stop=True)
                    h3 = hp.tile([H, CH], F32R, tag="h3", name=f"h3_{c}")
                    relu(h3[:], ps2[:], b2_c, on_act)

                    ps3 = psy.tile([1, CH], F32, tag="y", name=f"ps3_{c}")
                    for i in range(2):
                        nc.tensor.matmul(ps3[:, halves[i]],
                                         w3_c, h3[:, halves[i]],
                                         start=True, stop=True)
                    y1 = yp.tile([1, CH], F32, tag="y4", name=f"y1_{c}")
                    if on_act:
                        nc.scalar.activation(y1[:], ps3[:], COPY)
                    else:
                        nc.vector.tensor_copy(y1[:], ps3[:])
                    nc.sync.dma_start(y_d[c], y1[:])

    nc.compile()
    return nc


def _rne11(v):
    """Round fp32 to float32r's 11-bit mantissa (round-to-nearest-even).

    The PE consumes f32r as pre-rounded fp32 bits; feeding unrounded
    values produces garbage on HW (verified empirically: the casting DMA
    rounds exactly like this)."""
    b = np.ascontiguousarray(v, dtype=np.float32).view(np.uint32).astype(np.uint64)
    add = np.uint64(0x7FF) + ((b >> np.uint64(12)) & np.uint64(1))
    out = ((b + add) >> np.uint64(12) << np.uint64(12)).astype(np.uint32)
    return out.view(np.float32).reshape(np.asarray(v).shape)


def _route(x):
    """Per-point sub-model index, replicating the reference as executed.

    Uses the same jax ops on the same default device as the reference: on
    this backend the float->int32 convert rounds to nearest (not truncate),
    so cell components can be 2, making idx range over 0..14. Points with
    idx > 7 match no sub-model in the reference scan and keep y = 0.
    """
    import jax.numpy as jnp

    xj = jnp.asarray(np.ascontiguousarray(x, dtype=np.float32))
    g = jnp.asarray(np.array(GRID, dtype=np.float32))
    u = (xj + 1.0) / (2.0 + 1e-6)
    cell = np.asarray((u[:, ::-1] * g).astype(jnp.int32))
    idx = cell[:, 0] + cell[:, 1] * GRID[0] + cell[:, 2] * (GRID[0] * GRID[1])
    return idx


def _run_with_retry(nc, in_maps, trace):
    last = None
    for attempt in range(3):
        try:
            return run_bass_kernel_spmd(
                nc, in_maps, core_ids=list(range(NCORES)), trace=trace
            )
        except Exception as e:  # transient NRT_EXEC_UNIT_UNRECOVERABLE
            last = e
            time.sleep(15 * (attempt + 1))
    raise last


def kernel(x, W0, b0, W1, b1, W2, b2, W3, b3, mins, maxs, _trace=False):
    x = np.ascontiguousarray(x, dtype=np.float32)
    W0 = np.ascontiguousarray(W0, dtype=np.float32)
    b0 = np.ascontiguousarray(b0, dtype=np.float32)
    W1 = np.ascontiguousarray(W1, dtype=np.float32)
    b1 = np.ascontiguousarray(b1, dtype=np.float32)
    W2 = np.ascontiguousarray(W2, dtype=np.float32)
    b2 = np.ascontiguousarray(b2, dtype=np.float32)
    W3 = np.ascontiguousarray(W3, dtype=np.float32)
    b3 = np.ascontiguousarray(b3, dtype=np.float32)
    mins = np.ascontiguousarray(mins, dtype=np.float32)
    maxs = np.ascontiguousarray(maxs, dtype=np.float32)

    n = x.shape[0]
    idx = _route(x)
    valid = (idx >= 0) & (idx < M)

    order = np.argsort(np.where(valid, idx, M), kind="stable")
    counts = np.bincount(idx[valid], minlength=M)[:M]
    starts = np.concatenate([[0], np.cumsum(counts)])

    # Chop each expert's run of points into single-expert chunks of CH
    # points (last chunk of each expert padded), then deal chunks across
    # cores round-robin, largest first, for load balance.
    chunks = []  # (expert, sel_indices)
    for e in range(M):
        sel = order[starts[e] : starts[e + 1]]
        for o in range(0, len(sel), CH):
            chunks.append((e, sel[o : o + CH]))
    chunks.sort(key=lambda t: -len(t[1]))
    per_core = [chunks[c::NCORES] for c in range(NCORES)]
    n_chunks = max(len(pc) for pc in per_core)
    n_chunks = max(2, n_chunks + (n_chunks % 2))  # even, >= 2

    w0aug = np.concatenate([W0, b0[:, None, :]], axis=1)  # [M, 4, H]
    scale = np.float32(2.0) / (maxs - mins)  # [M, 3]

    in_maps = []
    placements = []
    for c in range(NCORES):
        xw = np.zeros((n_chunks, 4, CH + H), dtype=np.float32)
        pw = np.zeros((n_chunks, H, 259), dtype=np.float32)
        place = []
        for s, (e, sel) in enumerate(per_core[c]):
            xn = np.float32(-1.0) + (x[sel] - mins[e]) * scale[e]
            xw[s, :3, : len(sel)] = xn.T
            xw[s, 3, :CH] = 1.0
            xw[s, :, CH:] = w0aug[e]
            pw[s, :, 0:H] = W1[e]
            pw[s, :, H : 2 * H] = W2[e]
            pw[s, :, 256] = b1[e]
            pw[s, :, 257] = b2[e]
            pw[s, :, 258] = W3[e][:, 0]
            place.append((e, sel, s))
        placements.append(place)
        in_maps.append({"xw": _rne11(xw), "pw": _rne11(pw)})

    if n_chunks not in _programs:
        _programs[n_chunks] = _build_program(n_chunks)
    nc = _programs[n_chunks]

    res = _run_with_retry(nc, in_maps, _trace)

    y = np.zeros((n, 1), dtype=np.float32)
    for c in range(NCORES):
        yc = res.results[c]["y"].reshape(n_chunks, CH)
        for e, sel, s in placements[c]:
            y[sel, 0] = yc[s, : len(sel)] + b3[e, 0]

    if _trace:
        kernel._last_result = res
    return y


# revision 23
# speedup vs baseline: 1.2055x; 1.1632x over previous
"""Trainium2 Bass kernel for nn_Ensemble_SRN (MoE-routed ensemble of SRN MLPs).

Problem: N=131072 points in [-1,1]^3, a 2x2x2 grid of M=8 sub-model MLPs
(3 -> 128 -> 128 -> 128 -> 1, relu), each point evaluated by the sub-model
owning its grid cell (per the reference's routing as executed on this
backend, whose float->int32 convert rounds to nearest: points whose cell
index lands outside 0..7 match no sub-model and output 0).

Strategy: routed data-parallel with per-chunk expert weights. The host
computes each point's sub-model index with the same jax ops as the
reference (bit-identical routing), drops the no-expert points (y=0),
normalizes each point into its expert's local box, and packs the surviving
points into single-expert chunks of 1024 points (2 device tiles). Chunks
are dealt round-robin across the 8 NeuronCores for near-perfect load
balance (expert loads are highly skewed by the rounding behavior). Each
chunk's weight set streams from DRAM alongside its points, so any core can
process any mix of experts.

Device program (per core, identical SPMD, T tiles = T/2 chunks):
  per chunk (one weight set, two 512-pt tiles in [128, 1024] buffers):
  - L0: psum = W0aug.T @ xT (K=4, ones row folds b0) -> relu -> h1 (f32r)
  - L1: psum = W1.T @ h1 (K=128) -> relu+b1 -> h2
  - L2: psum = W2.T @ h2 (K=128) -> relu+b2 -> h3
  - L3: psum = W3.T @ h3 (M=1) col-packed: the 4 y-rows of a 4-tile group
    land on partitions 0/32/64/96 of one PSUM bank, drained by one
    partition-strided copy, then DMA'd out.
Relus alternate between the Scalar(ACT) and Vector(DVE) engines. Matmuls
run in float32r (full PE row rate, ~1.5e-4 relative error vs fp32).
b3 is added on the host during the gather.
"""

import time

import numpy as np

import concourse.bass as bass
import concourse.mybir as mybir
import concourse.tile as tile
from concourse import bacc
from concourse.bass_utils import run_bass_kernel_spmd

N = 131072
M = 8
H = 128
GRID = (2, 2, 2)
NCORES = 8

PT = 512       # points per tile
CH = 2 * PT    # points per single-expert chunk

F32 = mybir.dt.float32
F32R = mybir.dt.float32r
RELU = mybir.ActivationFunctionType.Relu
COPY = mybir.ActivationFunctionType.Copy
BF16 = mybir.dt.bfloat16
ALU_ADD = mybir.AluOpType.add
ALU_MAX = mybir.AluOpType.max

_programs = {}  # n_chunks -> compiled Bacc


def _build_program(NC_CHUNKS):
    """NC_CHUNKS single-expert chunks of CH points; must be even (groups of
    two chunks share one y PSUM bank via col-packing)."""
    assert NC_CHUNKS % 2 == 0
    nc = bacc.Bacc("TRN2", target_bir_lowering=False, debug=False)

    # xw: per-chunk [4, CH + H]: cols 0:CH = points x^T (3 coords + ones),
    # cols CH: = W0aug (K=4 partition dim shared with x).
    xw_d = nc.dram_tensor("xw", [NC_CHUNKS, 4, CH + H], F32R,
                          kind="ExternalInput")
    # pw: per-chunk packed [H, 259]: 0:128 W1 | 128:256 W2 | 256 b1 |
    # 257 b2 | 258 w3.
    pw_d = nc.dram_tensor("pw", [NC_CHUNKS, H, 259], F32R,
                          kind="ExternalInput")
    y_d = nc.dram_tensor("y", [NC_CHUNKS, CH], F32, kind="ExternalOutput")

    halves = (slice(0, PT), slice(PT, CH))

    with tile.TileContext(nc) as tc:
        with (
            tc.tile_pool(name="xp", bufs=6) as xp,
            tc.tile_pool(name="wp", bufs=6) as wp,
            tc.tile_pool(name="bp", bufs=4) as bp,
            tc.tile_pool(name="hp", bufs=4) as hp,
            tc.tile_pool(name="yp", bufs=2) as yp,
            tc.tile_pool(name="psm", bufs=6, space=bass.MemorySpace.PSUM) as psm,
            tc.tile_pool(name="psy", bufs=1, space=bass.MemorySpace.PSUM) as psy,
        ):
            def relu(out, ps, b, on_act):
                if on_act:
                    nc.scalar.activation(out, ps, RELU,
                                         bias=0.0 if b is None else b)
                else:
                    nc.vector.tensor_scalar(out, ps,
                                            0.0 if b is None else b,
                                            0.0, ALU_ADD, ALU_MAX)

            xw_c0 = xp.tile([4, CH + H], F32R, tag="warmx")
            nc.gpsimd.dma_start(xw_c0[:], xw_d[0])
            # HAM warmup: ~3.5us of back-to-back matmuls lifts the PE
            # clock gate to 8/8 (2.4 GHz) before the real stream begins.
            warm_ps = psm.tile([H, PT], F32, tag="mm", name="warm_ps")
            for wi in range(9):
                nc.tensor.matmul(warm_ps[:], xw_c0[:, CH : CH + H],
                                 xw_c0[:, 0:PT], start=True, stop=True,
                                 skip_group_check=(wi > 0))
            for g in range(NC_CHUNKS // 2):
                for ci in range(2):
                    c = 2 * g + ci
                    xw_c = xp.tile([4, CH + H], F32R, tag="x")
                    nc.gpsimd.dma_start(xw_c[:], xw_d[c])
                    pw_c = wp.tile([H, 259], F32R, tag="pw")
                    nc.sync.dma_start(pw_c[:], pw_d[c])
                    x_c = xw_c[:, 0:CH]
                    w0_c = xw_c[:, CH : CH + H]
                    w1_c = pw_c[:, 0:H]
                    w2_c = pw_c[:, H : 2 * H]
                    b1_c = pw_c[:, 256:257].bitcast(F32)
                    b2_c = pw_c[:, 257:258].bitcast(F32)
                    w3_c = pw_c[:, 258:259]

                    on_act = c % 2 == 0

                    h1 = hp.tile([H, CH], F32R, tag="h1", name=f"h1_{c}")
                    for i in range(2):
                        psx = psm.tile([H, PT], F32, tag="mm",
                                       name=f"ps0_{c}_{i}")
                        nc.tensor.matmul(psx[:], w0_c, x_c[:, halves[i]],
                                         start=True, stop=True)
                        relu(h1[:, halves[i]], psx[:], None, i == 0)

                    h2 = hp.tile([H, CH], F32R, tag="h2", name=f"h2_{c}")
                    for i in range(2):
                        psx = psm.tile([H, PT], F32, tag="mm",
                                       name=f"ps1_{c}_{i}")
                        nc.tensor.matmul(psx[:], w1_c, h1[:, halves[i]],
                                         start=True, stop=True)
                        relu(h2[:, halves[i]], psx[:], b1_c, i == 1)

                    h3 = hp.tile([H, CH], F32R, tag="h3", name=f"h3_{c}")
                    for i in range(2):
                        psx = psm.tile([H, PT], F32, tag="mm",
                                       name=f"ps2_{c}_{i}")
                        nc.tensor.matmul(psx[:], w2_c, h2[:, halves[i]],
                                         start=True, stop=True)
                        relu(h3[:, halves[i]], psx[:], b2_c, i == 0)

                    ps3 = psy.tile([1, CH], F32, tag="y", name=f"ps3_{c}")
                    for i in range(2):
                        nc.tensor.matmul(ps3[:, halves[i]],
                                         w3_c, h3[:, halves[i]],
                                         start=True, stop=True)
                    y1 = yp.tile([1, CH], F32, tag="y4", name=f"y1_{c}")
                    if on_act:
                        nc.scalar.activation(y1[:], ps3[:], COPY)
                    else:
                        nc.vector.tensor_copy(y1[:], ps3[:])
                    nc.sync.dma_start(y_d[c], y1[:])

    nc.compile()
    return nc


def _rne11(v):
    """Round fp32 to float32r's 11-bit mantissa (round-to-nearest-even).

    The PE consumes f32r as pre-rounded fp32 bits; feeding unrounded
    values produces garbage on HW (verified empirically: the casting DMA
    rounds exactly like this)."""
    b = np.ascontiguousarray(v, dtype=np.float32).view(np.uint32).astype(np.uint64)
    add = np.uint64(0x7FF) + ((b >> np.uint64(12)) & np.uint64(1))
    out = ((b + add) >> np.uint64(12) << np.uint64(12)).astype(np.uint32)
    return out.view(np.float32).reshape(np.asarray(v).shape)


def _route(x):
    """Per-point sub-model index, replicating the reference as executed.

    Uses the same jax ops on the same default device as the reference: on
    this backend the float->int32 convert rounds to nearest (not truncate),
    so cell components can be 2, making idx range over 0..14. Points with
    idx > 7 match no sub-model in the reference scan and keep y = 0.
    """
    import jax.numpy as jnp

    xj = jnp.asarray(np.ascontiguousarray(x, dtype=np.float32))
    g = jnp.asarray(np.array(GRID, dtype=np.float32))
    u = (xj + 1.0) / (2.0 + 1e-6)
    cell = np.asarray((u[:, ::-1] * g).astype(jnp.int32))
    idx = cell[:, 0] + cell[:, 1] * GRID[0] + cell[:, 2] * (GRID[0] * GRID[1])
    return idx


def _run_with_retry(nc, in_maps, trace):
    last = None
    for attempt in range(3):
        try:
            return run_bass_kernel_spmd(
                nc, in_maps, core_ids=list(range(NCORES)), trace=trace
            )
        except Exception as e:  # transient NRT_EXEC_UNIT_UNRECOVERABLE
            last = e
            time.sleep(15 * (attempt + 1))
    raise last


def kernel(x, W0, b0, W1, b1, W2, b2, W3, b3, mins, maxs, _trace=False):
    x = np.ascontiguousarray(x, dtype=np.float32)
    W0 = np.ascontiguousarray(W0, dtype=np.float32)
    b0 = np.ascontiguousarray(b0, dtype=np.float32)
    W1 = np.ascontiguousarray(W1, dtype=np.float32)
    b1 = np.ascontiguousarray(b1, dtype=np.float32)
    W2 = np.ascontiguousarray(W2, dtype=np.float32)
    b2 = np.ascontiguousarray(b2, dtype=np.float32)
    W3 = np.ascontiguousarray(W3, dtype=np.float32)
    b3 = np.ascontiguousarray(b3, dtype=np.float32)
    mins = np.ascontiguousarray(mins, dtype=np.float32)
    maxs = np.ascontiguousarray(maxs, dtype=np.float32)

    n = x.shape[0]
    idx = _route(x)
    valid = (idx >= 0) & (idx < M)

    order = np.argsort(np.where(valid, idx, M), kind="stable")
    counts = np.bincount(idx[valid], minlength=M)[:M]
    starts = np.concatenate([[0], np.cumsum(counts)])

    # Chop each expert's run of points into single-expert chunks of CH
    # points (last chunk of each expert padded), then deal chunks across
    # cores round-robin, largest first, for load balance.
    chunks = []  # (expert, sel_indices)
    for e in range(M):
        sel = order[starts[e] : starts[e + 1]]
        for o in range(0, len(sel), CH):
            chunks.append((e, sel[o : o + CH]))
    chunks.sort(key=lambda t: -len(t[1]))
    per_core = [chunks[c::NCORES] for c in range(NCORES)]
    n_chunks = max(len(pc) for pc in per_core)
    n_chunks = max(2, n_chunks + (n_chunks % 2))  # even, >= 2

    w0aug = np.concatenate([W0, b0[:, None, :]], axis=1)  # [M, 4, H]
    scale = np.float32(2.0) / (maxs - mins)  # [M, 3]

    in_maps = []
    placements = []
    for c in range(NCORES):
        xw = np.zeros((n_chunks, 4, CH + H), dtype=np.float32)
        pw = np.zeros((n_chunks, H, 259), dtype=np.float32)
        place = []
        for s, (e, sel) in enumerate(per_core[c]):
            xn = np.float32(-1.0) + (x[sel] - mins[e]) * scale[e]
            xw[s, :3, : len(sel)] = xn.T
            xw[s, 3, :CH] = 1.0
            xw[s, :, CH:] = w0aug[e]
            pw[s, :, 0:H] = W1[e]
            pw[s, :, H : 2 * H] = W2[e]
            pw[s, :, 256] = b1[e]
            pw[s, :, 257] = b2[e]
            pw[s, :, 258] = W3[e][:, 0]
            place.append((e, sel, s))
        placements.append(place)
        in_maps.append({"xw": _rne11(xw), "pw": _rne11(pw)})

    if n_chunks not in _programs:
        _programs[n_chunks] = _build_program(n_chunks)
    nc = _programs[n_chunks]

    res = _run_with_retry(nc, in_maps, _trace)

    y = np.zeros((n, 1), dtype=np.float32)
    for c in range(NCORES):
        yc = res.results[c]["y"].reshape(n_chunks, CH)
        for e, sel, s in placements[c]:
            y[sel, 0] = yc[s, : len(sel)] + b3[e, 0]

    if _trace:
        kernel._last_result = res
    return y


# revision 24
# speedup vs baseline: 1.2244x; 1.0157x over previous
"""Trainium2 Bass kernel for nn_Ensemble_SRN (MoE-routed ensemble of SRN MLPs).

Problem: N=131072 points in [-1,1]^3, a 2x2x2 grid of M=8 sub-model MLPs
(3 -> 128 -> 128 -> 128 -> 1, relu), each point evaluated by the sub-model
owning its grid cell (per the reference's routing as executed on this
backend, whose float->int32 convert rounds to nearest: points whose cell
index lands outside 0..7 match no sub-model and output 0).

Strategy: routed data-parallel with per-chunk expert weights. The host
computes each point's sub-model index with the same jax ops as the
reference (bit-identical routing), drops the no-expert points (y=0),
normalizes each point into its expert's local box, and packs the surviving
points into single-expert chunks of 1024 points (2 device tiles). Chunks
are dealt round-robin across the 8 NeuronCores for near-perfect load
balance (expert loads are highly skewed by the rounding behavior). Each
chunk's weight set streams from DRAM alongside its points, so any core can
process any mix of experts.

Device program (per core, identical SPMD, T tiles = T/2 chunks):
  per chunk (one weight set, two 512-pt tiles in [128, 1024] buffers):
  - L0: psum = W0aug.T @ xT (K=4, ones row folds b0) -> relu -> h1 (f32r)
  - L1: psum = W1.T @ h1 (K=128) -> relu+b1 -> h2
  - L2: psum = W2.T @ h2 (K=128) -> relu+b2 -> h3
  - L3: psum = W3.T @ h3 (M=1) col-packed: the 4 y-rows of a 4-tile group
    land on partitions 0/32/64/96 of one PSUM bank, drained by one
    partition-strided copy, then DMA'd out.
Relus alternate between the Scalar(ACT) and Vector(DVE) engines. Matmuls
run in float32r (full PE row rate, ~1.5e-4 relative error vs fp32).
b3 is added on the host during the gather.
"""

import time

import numpy as np

import concourse.bass as bass
import concourse.mybir as mybir
import concourse.tile as tile
from concourse import bacc
from concourse.bass_utils import run_bass_kernel_spmd

N = 131072
M = 8
H = 128
GRID = (2, 2, 2)
NCORES = 8

PT = 512       # points per tile
CH = 2 * PT    # points per single-expert chunk

F32 = mybir.dt.float32
F32R = mybir.dt.float32r
RELU = mybir.ActivationFunctionType.Relu
COPY = mybir.ActivationFunctionType.Copy
BF16 = mybir.dt.bfloat16
ALU_ADD = mybir.AluOpType.add
ALU_MAX = mybir.AluOpType.max

_programs = {}  # n_chunks -> compiled Bacc


def _build_program(NC_CHUNKS):
    """NC_CHUNKS single-expert chunks of CH points; must be even (groups of
    two chunks share one y PSUM bank via col-packing)."""
    assert NC_CHUNKS % 2 == 0
    nc = bacc.Bacc("TRN2", target_bir_lowering=False, debug=False)

    # xw: per-chunk [4, CH + H]: cols 0:CH = points x^T (3 coords + ones),
    # cols CH: = W0aug (K=4 partition dim shared with x).
    xw_d = nc.dram_tensor("xw", [NC_CHUNKS, 4, CH + H], F32R,
                          kind="ExternalInput")
    # pw: per-chunk packed [H, 259]: 0:128 W1 | 128:256 W2 | 256 b1 |
    # 257 b2 | 258 w3.
    pw_d = nc.dram_tensor("pw", [NC_CHUNKS, H, 259], F32R,
                          kind="ExternalInput")
    y_d = nc.dram_tensor("y", [NC_CHUNKS, CH], F32, kind="ExternalOutput")

    halves = (slice(0, PT), slice(PT, CH))

    with tile.TileContext(nc) as tc:
        with (
            tc.tile_pool(name="xp", bufs=6) as xp,
            tc.tile_pool(name="wp", bufs=6) as wp,
            tc.tile_pool(name="bp", bufs=4) as bp,
            tc.tile_pool(name="hp", bufs=4) as hp,
            tc.tile_pool(name="yp", bufs=2) as yp,
            tc.tile_pool(name="psm", bufs=6, space=bass.MemorySpace.PSUM) as psm,
            tc.tile_pool(name="psy", bufs=1, space=bass.MemorySpace.PSUM) as psy,
        ):
            def relu(out, ps, b, on_act):
                if on_act:
                    nc.scalar.activation(out, ps, RELU,
                                         bias=0.0 if b is None else b)
                else:
                    nc.vector.tensor_scalar(out, ps,
                                            0.0 if b is None else b,
                                            0.0, ALU_ADD, ALU_MAX)

            pw_c0 = wp.tile([H, 259], F32R, tag="pw", name="pw_warm")
            nc.sync.dma_start(pw_c0[:], pw_d[0])
            # HAM warmup: >3.4us of gap-free K=128 matmuls lifts the PE
            # clock gate to 8/8 (2.4 GHz) before the real stream begins.
            # (K must be large: activity scales with active rows, so K=4
            # warmup matmuls never trip the busy window.)
            warm_ps = psm.tile([H, PT], F32, tag="mm", name="warm_ps")
            for wi in range(18):
                nc.tensor.matmul(warm_ps[:, 0:256], pw_c0[:, 0:H],
                                 pw_c0[:, 0:256], start=True, stop=True,
                                 skip_group_check=(wi > 0))
            for g in range(NC_CHUNKS // 2):
                for ci in range(2):
                    c = 2 * g + ci
                    xw_c = xp.tile([4, CH + H], F32R, tag="x")
                    nc.gpsimd.dma_start(xw_c[:], xw_d[c])
                    pw_c = wp.tile([H, 259], F32R, tag="pw")
                    nc.sync.dma_start(pw_c[:], pw_d[c])
                    x_c = xw_c[:, 0:CH]
                    w0_c = xw_c[:, CH : CH + H]
                    w1_c = pw_c[:, 0:H]
                    w2_c = pw_c[:, H : 2 * H]
                    b1_c = pw_c[:, 256:257].bitcast(F32)
                    b2_c = pw_c[:, 257:258].bitcast(F32)
                    w3_c = pw_c[:, 258:259]

                    on_act = c % 2 == 0

                    h1 = hp.tile([H, CH], F32R, tag="h1", name=f"h1_{c}")
                    for i in range(2):
                        psx = psm.tile([H, PT], F32, tag="mm",
                                       name=f"ps0_{c}_{i}")
                        nc.tensor.matmul(psx[:], w0_c, x_c[:, halves[i]],
                                         start=True, stop=True)
                        relu(h1[:, halves[i]], psx[:], None, i == 0)

                    h2 = hp.tile([H, CH], F32R, tag="h2", name=f"h2_{c}")
                    for i in range(2):
                        psx = psm.tile([H, PT], F32, tag="mm",
                                       name=f"ps1_{c}_{i}")
                        nc.tensor.matmul(psx[:], w1_c, h1[:, halves[i]],
                                         start=True, stop=True)
                        relu(h2[:, halves[i]], psx[:], b1_c, i == 1)

                    h3 = hp.tile([H, CH], F32R, tag="h3", name=f"h3_{c}")
                    for i in range(2):
                        psx = psm.tile([H, PT], F32, tag="mm",
                                       name=f"ps2_{c}_{i}")
                        nc.tensor.matmul(psx[:], w2_c, h2[:, halves[i]],
                                         start=True, stop=True)
                        relu(h3[:, halves[i]], psx[:], b2_c, i == 0)

                    ps3 = psy.tile([1, CH], F32, tag="y", name=f"ps3_{c}")
                    for i in range(2):
                        nc.tensor.matmul(ps3[:, halves[i]],
                                         w3_c, h3[:, halves[i]],
                                         start=True, stop=True)
                    y1 = yp.tile([1, CH], F32, tag="y4", name=f"y1_{c}")
                    if on_act:
                        nc.scalar.activation(y1[:], ps3[:], COPY)
                    else:
                        nc.vector.tensor_copy(y1[:], ps3[:])
                    nc.sync.dma_start(y_d[c], y1[:])

    nc.compile()
    return nc


def _rne11(v):
    """Round fp32 to float32r's 11-bit mantissa (round-to-nearest-even).

    The PE consumes f32r as pre-rounded fp32 bits; feeding unrounded
    values produces garbage on HW (verified empirically: the casting DMA
    rounds exactly like this)."""
    b = np.ascontiguousarray(v, dtype=np.float32).view(np.uint32).astype(np.uint64)
    add = np.uint64(0x7FF) + ((b >> np.uint64(12)) & np.uint64(1))
    out = ((b + add) >> np.uint64(12) << np.uint64(12)).astype(np.uint32)
    return out.view(np.float32).reshape(np.asarray(v).shape)


def _route(x):
    """Per-point sub-model index, replicating the reference as executed.

    Uses the same jax ops on the same default device as the reference: on
    this backend the float->int32 convert rounds to nearest (not truncate),
    so cell components can be 2, making idx range over 0..14. Points with
    idx > 7 match no sub-model in the reference scan and keep y = 0.
    """
    import jax.numpy as jnp

    xj = jnp.asarray(np.ascontiguousarray(x, dtype=np.float32))
    g = jnp.asarray(np.array(GRID, dtype=np.float32))
    u = (xj + 1.0) / (2.0 + 1e-6)
    cell = np.asarray((u[:, ::-1] * g).astype(jnp.int32))
    idx = cell[:, 0] + cell[:, 1] * GRID[0] + cell[:, 2] * (GRID[0] * GRID[1])
    return idx


def _run_with_retry(nc, in_maps, trace):
    last = None
    for attempt in range(3):
        try:
            return run_bass_kernel_spmd(
                nc, in_maps, core_ids=list(range(NCORES)), trace=trace
            )
        except Exception as e:  # transient NRT_EXEC_UNIT_UNRECOVERABLE
            last = e
            time.sleep(15 * (attempt + 1))
    raise last


def kernel(x, W0, b0, W1, b1, W2, b2, W3, b3, mins, maxs, _trace=False):
    x = np.ascontiguousarray(x, dtype=np.float32)
    W0 = np.ascontiguousarray(W0, dtype=np.float32)
    b0 = np.ascontiguousarray(b0, dtype=np.float32)
    W1 = np.ascontiguousarray(W1, dtype=np.float32)
    b1 = np.ascontiguousarray(b1, dtype=np.float32)
    W2 = np.ascontiguousarray(W2, dtype=np.float32)
    b2 = np.ascontiguousarray(b2, dtype=np.float32)
    W3 = np.ascontiguousarray(W3, dtype=np.float32)
    b3 = np.ascontiguousarray(b3, dtype=np.float32)
    mins = np.ascontiguousarray(mins, dtype=np.float32)
    maxs = np.ascontiguousarray(maxs, dtype=np.float32)

    n = x.shape[0]
    idx = _route(x)
    valid = (idx >= 0) & (idx < M)

    order = np.argsort(np.where(valid, idx, M), kind="stable")
    counts = np.bincount(idx[valid], minlength=M)[:M]
    starts = np.concatenate([[0], np.cumsum(counts)])

    # Chop each expert's run of points into single-expert chunks of CH
    # points (last chunk of each expert padded), then deal chunks across
    # cores round-robin, largest first, for load balance.
    chunks = []  # (expert, sel_indices)
    for e in range(M):
        sel = order[starts[e] : starts[e + 1]]
        for o in range(0, len(sel), CH):
            chunks.append((e, sel[o : o + CH]))
    chunks.sort(key=lambda t: -len(t[1]))
    per_core = [chunks[c::NCORES] for c in range(NCORES)]
    n_chunks = max(len(pc) for pc in per_core)
    n_chunks = max(2, n_chunks + (n_chunks % 2))  # even, >= 2

    w0aug = np.concatenate([W0, b0[:, None, :]], axis=1)  # [M, 4, H]
    scale = np.float32(2.0) / (maxs - mins)  # [M, 3]

    in_maps = []
    placements = []
    for c in range(NCORES):
        xw = np.zeros((n_chunks, 4, CH + H), dtype=np.float32)
        pw = np.zeros((n_chunks, H, 259), dtype=np.float32)
        place = []
        for s, (e, sel) in enumerate(per_core[c]):
            xn = np.float32(-1.0) + (x[sel] - mins[e]) * scale[e]
            xw[s, :3, : len(sel)] = xn.T
            xw[s, 3, :CH] = 1.0
            xw[s, :, CH:] = w0aug[e]
            pw[s, :, 0:H] = W1[e]
            pw[s, :, H : 2 * H] = W2[e]
            pw[s, :, 256] = b1[e]
            pw[s, :, 257] = b2[e]
            pw[s, :, 258] = W3[e][:, 0]
            place.append((e, sel, s))
        placements.append(place)
        in_maps.append({"xw": _rne11(xw), "pw": _rne11(pw)})

    if n_chunks not in _programs:
        _programs[n_chunks] = _build_program(n_chunks)
    nc = _programs[n_chunks]

    res = _run_with_retry(nc, in_maps, _trace)

    y = np.zeros((n, 1), dtype=np.float32)
    for c in range(NCORES):
        yc = res.results[c]["y"].reshape(n_chunks, CH)
        for e, sel, s in placements[c]:
            y[sel, 0] = yc[s, : len(sel)] + b3[e, 0]

    if _trace:
        kernel._last_result = res
    return y


# revision 27
# speedup vs baseline: 1.7299x; 1.4129x over previous
"""Trainium2 Bass kernel for nn_Ensemble_SRN (MoE-routed ensemble of SRN MLPs).

Problem: N=131072 points in [-1,1]^3, a 2x2x2 grid of M=8 sub-model MLPs
(3 -> 128 -> 128 -> 128 -> 1, relu), each point evaluated by the sub-model
owning its grid cell (per the reference's routing as executed on this
backend, whose float->int32 convert rounds to nearest: points whose cell
index lands outside 0..7 match no sub-model and output 0).

Strategy: routed data-parallel with per-chunk expert weights. The host
computes each point's sub-model index with the same jax ops as the
reference (bit-identical routing), drops the no-expert points (y=0),
normalizes each point into its expert's local box, and packs the surviving
points into single-expert chunks of 1024 points (2 device tiles). Chunks
are dealt round-robin across the 8 NeuronCores for near-perfect load
balance (expert loads are highly skewed by the rounding behavior). Each
chunk's weight set streams from DRAM alongside its points, so any core can
process any mix of experts.

Device program (per core, identical SPMD, T tiles = T/2 chunks):
  per chunk (one weight set, two 512-pt tiles in [128, 1024] buffers):
  - L0: psum = W0aug.T @ xT (K=4, ones row folds b0) -> relu -> h1 (f32r)
  - L1: psum = W1.T @ h1 (K=128) -> relu+b1 -> h2
  - L2: psum = W2.T @ h2 (K=128) -> relu+b2 -> h3
  - L3: psum = W3.T @ h3 (M=1) col-packed: the 4 y-rows of a 4-tile group
    land on partitions 0/32/64/96 of one PSUM bank, drained by one
    partition-strided copy, then DMA'd out.
Relus alternate between the Scalar(ACT) and Vector(DVE) engines. Matmuls
run in float32r (full PE row rate, ~1.5e-4 relative error vs fp32).
b3 is added on the host during the gather.
"""

import time

import numpy as np

import concourse.bass as bass
import concourse.mybir as mybir
import concourse.tile as tile
from concourse import bacc
from concourse.bass_utils import run_bass_kernel_spmd

N = 131072
M = 8
H = 128
GRID = (2, 2, 2)
NCORES = 8

PT = 512       # points per tile
CH = 2 * PT    # points per single-expert chunk

F32 = mybir.dt.float32
F32R = mybir.dt.float32r
RELU = mybir.ActivationFunctionType.Relu
COPY = mybir.ActivationFunctionType.Copy
BF16 = mybir.dt.bfloat16
ALU_ADD = mybir.AluOpType.add
ALU_MAX = mybir.AluOpType.max

_programs = {}  # n_chunks -> compiled Bacc


def _build_program(NC_CHUNKS):
    """NC_CHUNKS single-expert chunks of CH points; must be even (groups of
    two chunks share one y PSUM bank via col-packing)."""
    assert NC_CHUNKS % 2 == 0
    nc = bacc.Bacc("TRN2", target_bir_lowering=False, debug=False)

    # xw: per-chunk [4, CH + H]: cols 0:CH = points x^T (3 coords + ones),
    # cols CH: = W0aug (K=4 partition dim shared with x).
    xw_d = nc.dram_tensor("xw", [NC_CHUNKS, 4, CH + H], F32R,
                          kind="ExternalInput")
    # pw: per-chunk packed [H, 259]: 0:128 W1 | 128:256 W2 | 256 b1 |
    # 257 b2 | 258 w3.
    pw_d = nc.dram_tensor("pw", [NC_CHUNKS, H, 259], F32R,
                          kind="ExternalInput")
    y_d = nc.dram_tensor("y", [NC_CHUNKS, CH], F32, kind="ExternalOutput")

    halves = (slice(0, PT), slice(PT, CH))

    with tile.TileContext(nc) as tc:
        with (
            tc.tile_pool(name="xp", bufs=6) as xp,
            tc.tile_pool(name="wp", bufs=6) as wp,
            tc.tile_pool(name="hp", bufs=6) as hp,
            tc.tile_pool(name="yp", bufs=4) as yp,
            tc.tile_pool(name="psm", bufs=4, space=bass.MemorySpace.PSUM) as psm,
            tc.tile_pool(name="psy", bufs=2, space=bass.MemorySpace.PSUM) as psy,
        ):
            def relu(out, ps, b, on_act):
                if on_act:
                    nc.scalar.activation(out, ps, RELU,
                                         bias=0.0 if b is None else b)
                else:
                    nc.vector.tensor_scalar(out, ps,
                                            0.0 if b is None else b,
                                            0.0, ALU_ADD, ALU_MAX)

            # DMA all chunk inputs (x on the gpsimd queue, weights on sync)
            xw_cs, pw_cs = [], []
            for c in range(NC_CHUNKS):
                xw_c = xp.tile([4, CH + H], F32R, tag="x", name=f"xw_{c}")
                nc.gpsimd.dma_start(xw_c[:], xw_d[c])
                pw_c = wp.tile([H, 259], F32R, tag="pw", name=f"pw_{c}")
                nc.sync.dma_start(pw_c[:], pw_d[c])
                xw_cs.append(xw_c)
                pw_cs.append(pw_c)

            # HAM warmup: >3.4us of gap-free K=128 matmuls lifts the PE
            # clock gate to 8/8 (2.4 GHz) before the real stream begins.
            # (K must be large: activity scales with active rows, so K=4
            # warmup matmuls never trip the busy window.)
            warm_ps = psm.tile([H, PT], F32, tag="mm", name="warm_ps")
            for wi in range(18):
                nc.tensor.matmul(warm_ps[:, 0:256], pw_cs[0][:, 0:H],
                                 pw_cs[0][:, 0:256], start=True, stop=True,
                                 skip_group_check=(wi > 0))

            # Layer-major streams in blocks of B chunks: a block's L0
            # matmuls run back-to-back (dense PE stream stays HAM-warm),
            # relus trail on ACT/DVE. B <= pool bufs avoids slot deadlock.
            B = 6
            par = 0
            for b0 in range(0, NC_CHUNKS, B):
                blk = range(b0, min(b0 + B, NC_CHUNKS))
                h1s, h2s, h3s = {}, {}, {}
                for c in blk:
                    h1 = hp.tile([H, CH], F32R, tag="h1", name=f"h1_{c}")
                    h1s[c] = h1
                    for i in range(2):
                        psx = psm.tile([H, PT], F32, tag="mm",
                                       name=f"ps0_{c}_{i}")
                        nc.tensor.matmul(psx[:], xw_cs[c][:, CH : CH + H],
                                         xw_cs[c][:, 0:CH][:, halves[i]],
                                         start=True, stop=True)
                        relu(h1[:, halves[i]], psx[:], None, par % 2 == 0)
                        par += 1
                for c in blk:
                    h2 = hp.tile([H, CH], F32R, tag="h2", name=f"h2_{c}")
                    h2s[c] = h2
                    for i in range(2):
                        psx = psm.tile([H, PT], F32, tag="mm",
                                       name=f"ps1_{c}_{i}")
                        nc.tensor.matmul(psx[:], pw_cs[c][:, 0:H],
                                         h1s[c][:, halves[i]],
                                         start=True, stop=True)
                        relu(h2[:, halves[i]], psx[:],
                             pw_cs[c][:, 256:257].bitcast(F32), par % 2 == 0)
                        par += 1
                for c in blk:
                    h3 = hp.tile([H, CH], F32R, tag="h3", name=f"h3_{c}")
                    h3s[c] = h3
                    for i in range(2):
                        psx = psm.tile([H, PT], F32, tag="mm",
                                       name=f"ps2_{c}_{i}")
                        nc.tensor.matmul(psx[:], pw_cs[c][:, H : 2 * H],
                                         h2s[c][:, halves[i]],
                                         start=True, stop=True)
                        relu(h3[:, halves[i]], psx[:],
                             pw_cs[c][:, 257:258].bitcast(F32), par % 2 == 0)
                        par += 1
                for c in blk:
                    ps3 = psy.tile([1, CH], F32, tag="y", name=f"ps3_{c}")
                    for i in range(2):
                        nc.tensor.matmul(ps3[:, halves[i]],
                                         pw_cs[c][:, 258:259],
                                         h3s[c][:, halves[i]],
                                         start=True, stop=True)
                    y1 = yp.tile([1, CH], F32, tag="y4", name=f"y1_{c}")
                    if c % 2 == 0:
                        nc.scalar.activation(y1[:], ps3[:], COPY)
                    else:
                        nc.vector.tensor_copy(y1[:], ps3[:])
                    nc.sync.dma_start(y_d[c], y1[:])

    nc.compile()
    return nc


def _rne11(v):
    """Round fp32 to float32r's 11-bit mantissa (round-to-nearest-even).

    The PE consumes f32r as pre-rounded fp32 bits; feeding unrounded
    values produces garbage on HW (verified empirically: the casting DMA
    rounds exactly like this)."""
    b = np.ascontiguousarray(v, dtype=np.float32).view(np.uint32).astype(np.uint64)
    add = np.uint64(0x7FF) + ((b >> np.uint64(12)) & np.uint64(1))
    out = ((b + add) >> np.uint64(12) << np.uint64(12)).astype(np.uint32)
    return out.view(np.float32).reshape(np.asarray(v).shape)


def _route(x):
    """Per-point sub-model index, replicating the reference as executed.

    Uses the same jax ops on the same default device as the reference: on
    this backend the float->int32 convert rounds to nearest (not truncate),
    so cell components can be 2, making idx range over 0..14. Points with
    idx > 7 match no sub-model in the reference scan and keep y = 0.
    """
    import jax.numpy as jnp

    xj = jnp.asarray(np.ascontiguousarray(x, dtype=np.float32))
    g = jnp.asarray(np.array(GRID, dtype=np.float32))
    u = (xj + 1.0) / (2.0 + 1e-6)
    cell = np.asarray((u[:, ::-1] * g).astype(jnp.int32))
    idx = cell[:, 0] + cell[:, 1] * GRID[0] + cell[:, 2] * (GRID[0] * GRID[1])
    return idx


def _run_with_retry(nc, in_maps, trace):
    last = None
    for attempt in range(3):
        try:
            return run_bass_kernel_spmd(
                nc, in_maps, core_ids=list(range(NCORES)), trace=trace
            )
        except Exception as e:  # transient NRT_EXEC_UNIT_UNRECOVERABLE
            last = e
            time.sleep(15 * (attempt + 1))
    raise last


def kernel(x, W0, b0, W1, b1, W2, b2, W3, b3, mins, maxs, _trace=False):
    x = np.ascontiguousarray(x, dtype=np.float32)
    W0 = np.ascontiguousarray(W0, dtype=np.float32)
    b0 = np.ascontiguousarray(b0, dtype=np.float32)
    W1 = np.ascontiguousarray(W1, dtype=np.float32)
    b1 = np.ascontiguousarray(b1, dtype=np.float32)
    W2 = np.ascontiguousarray(W2, dtype=np.float32)
    b2 = np.ascontiguousarray(b2, dtype=np.float32)
    W3 = np.ascontiguousarray(W3, dtype=np.float32)
    b3 = np.ascontiguousarray(b3, dtype=np.float32)
    mins = np.ascontiguousarray(mins, dtype=np.float32)
    maxs = np.ascontiguousarray(maxs, dtype=np.float32)

    n = x.shape[0]
    idx = _route(x)
    valid = (idx >= 0) & (idx < M)

    order = np.argsort(np.where(valid, idx, M), kind="stable")
    counts = np.bincount(idx[valid], minlength=M)[:M]
    starts = np.concatenate([[0], np.cumsum(counts)])

    # Chop each expert's run of points into single-expert chunks of CH
    # points (last chunk of each expert padded), then deal chunks across
    # cores round-robin, largest first, for load balance.
    chunks = []  # (expert, sel_indices)
    for e in range(M):
        sel = order[starts[e] : starts[e + 1]]
        for o in range(0, len(sel), CH):
            chunks.append((e, sel[o : o + CH]))
    chunks.sort(key=lambda t: -len(t[1]))
    per_core = [chunks[c::NCORES] for c in range(NCORES)]
    n_chunks = max(len(pc) for pc in per_core)
    n_chunks = max(2, n_chunks + (n_chunks % 2))  # even, >= 2

    w0aug = np.concatenate([W0, b0[:, None, :]], axis=1)  # [M, 4, H]
    scale = np.float32(2.0) / (maxs - mins)  # [M, 3]

    in_maps = []
    placements = []
    for c in range(NCORES):
        xw = np.zeros((n_chunks, 4, CH + H), dtype=np.float32)
        pw = np.zeros((n_chunks, H, 259), dtype=np.float32)
        place = []
        for s, (e, sel) in enumerate(per_core[c]):
            xn = np.float32(-1.0) + (x[sel] - mins[e]) * scale[e]
            xw[s, :3, : len(sel)] = xn.T
            xw[s, 3, :CH] = 1.0
            xw[s, :, CH:] = w0aug[e]
            pw[s, :, 0:H] = W1[e]
            pw[s, :, H : 2 * H] = W2[e]
            pw[s, :, 256] = b1[e]
            pw[s, :, 257] = b2[e]
            pw[s, :, 258] = W3[e][:, 0]
            place.append((e, sel, s))
        placements.append(place)
        in_maps.append({"xw": _rne11(xw), "pw": _rne11(pw)})

    if n_chunks not in _programs:
        _programs[n_chunks] = _build_program(n_chunks)
    nc = _programs[n_chunks]

    res = _run_with_retry(nc, in_maps, _trace)

    y = np.zeros((n, 1), dtype=np.float32)
    for c in range(NCORES):
        yc = res.results[c]["y"].reshape(n_chunks, CH)
        for e, sel, s in placements[c]:
            y[sel, 0] = yc[s, : len(sel)] + b3[e, 0]

    if _trace:
        kernel._last_result = res
    return y
